# revision 14
# baseline (speedup 1.0000x reference)
"""Trainium2 Bass kernel for the vq_codebook / HDC problem.

Math (reference):
    hv      = sign(feat @ proj_w.T)               [N=16384, D=10000], +-1 (0 -> +1)
    per_cls = segment_sum(hv, labels, K=3)        [3, D]
    updated = classify_weights + 0.5 * per_cls
    protos  = updated / max(||updated||_row, eps)
    logits  = hv @ protos.T                       [N, 3]

Strategy (8 NeuronCores, D-sharded, no collectives; ~150us HW vs 294us
baseline, rel err ~5e-3 vs 2e-2 gate):
  * Each core owns DLOC=1250 hyper-dims (10 d-tiles x 125 partitions), all
    N rows; host sorts rows by label so per-class sums become contiguous
    range sums along the free axis.
  * hv is stored as step(z) in {0,1} fp8e4 (16KB/partition per d-tile), so
    ALL 10 d-tiles stay SBUF-resident: feat is read exactly once per core
    and hv never touches HBM.  sign = 2*step - 1 is fixed up algebraically
    on the host.
  * Phase A (encode, ~103us, drain-bound): z = projw.T-tiles @ featT in
    [d, n] layout via fp16 matmuls (16-bit moving operand streams 1
    col/cycle vs ~2 for fp32r -- measured; fp16 input quantization costs
    ~4.8e-3 end-to-end rel err) into [125, 1024] PSUM window tiles, 4
    rotating 2-bank bufs so the fill<->drain WAR chain stays short.  One
    drain op per (d-tile, window) converts z -> step fp8 with fused
    per-segment accumulation (accum_out), split DVE (is_ge) / ACT
    (Sigmoid(2^20 z)) by a greedy balance on trace-fitted costs; both
    engines run concurrently on disjoint PSUM banks at ~1 elem/cyc/
    partition each -- the hard floor for this dataflow on trn2.
  * Phase B: u = (cw - 0.5*count) + step-sums == reference `updated`
    exactly; phase-C stationary = fp8(u/2) + fp8 residual per d-tile
    (zero-padded to 32 cols so every PSUM partition of a col group is
    written).  Keep-warm dummy matmuls bridge the PE idle gap so the HAM
    clock gate never re-throttles (3.4us window).
  * Phase C (consume, ~26us): P = (u_q/2) @ step via plain-fp8 matmuls
    packed 4x into the PE with column tiling (tile_position=(0,32g)): 4
    concurrent col-groups, each owning 2-3 d-tiles, stream their own
    moving operand -> ~79ns/matmul effective.  Drain halves split
    DVE/ACT; p_out DMAs alternate gpsimd/sync queues.
  * Host: logits = (4*P - rowsum(u_q)) / max(||u||, eps), un-permuted.
    u_q is re-quantized on host with ml_dtypes for an exact correction.
"""

import os
import sys

sys.path.insert(0, "/opt/trn_rl_repo")
os.environ.setdefault("MYCRO_LOCAL_CACHE", "1")

import numpy as np

import concourse.bass as bass
import concourse.tile as tile
from concourse import bacc
from concourse import mybir
from concourse.bass import MemorySpace
from concourse.bass_utils import run_bass_kernel_spmd

# ---------------------------------------------------------------- constants
N, C, D, K = 16384, 128, 10000, 3
NCORES = 8
DLOC = D // NCORES          # 1250 hyper-dims per core
PT = 125                    # partitions per d-tile
NT = DLOC // PT             # 10 d-tiles per core
G4 = 2048                   # feat DMA chunk
NG = N // G4                # 8 groups
WIN = 1024                  # phase-A window (2 PSUM banks; 4 bufs rotate)
NW = N // WIN               # 16 windows
MMC = 512                   # encode matmul chunk (PSUM bank = 512 fp32)
PC = 512                    # phase-C matmul out chunk (1 PSUM bank)
PCG = 2048                  # phase-C drain super-chunk
SIG_SCALE = 1048576.0       # 2^20: step(z) ~= Sigmoid(SIG_SCALE * z)
ACOLS = 20                  # accum columns reserved per d-tile

# phase-C column tiling: d-tile -> col group (3/3/2/2), round-robin issue
GID = [0, 0, 0, 1, 1, 1, 2, 2, 3, 3]
RR = [0, 3, 6, 8, 1, 4, 7, 9, 2, 5]
FIRST = {0: 0, 1: 3, 2: 6, 3: 8}
LAST = {0: 2, 1: 5, 2: 7, 3: 9}
SCW = 32                    # stationary cols (zero-padded past 2K so every
                            # PSUM partition in a col group is written)

MM_DT = mybir.dt.float16
FP8 = mybir.dt.float8e4
F32 = mybir.dt.float32
EPS = 1e-12

LAST_RESULTS = None         # BassKernelResults of the most recent run


def _subranges(cuts):
    """Ordered (w, s0, s1, cls) sub-ranges: each phase-A window [w*WIN,(w+1)*WIN)
    split at the sorted-label cut points so every range is single-class."""
    subs = []
    for w in range(NW):
        lo, hi = w * WIN, (w + 1) * WIN
        pts = [lo] + [c for c in cuts if lo < c < hi] + [hi]
        for a, b in zip(pts[:-1], pts[1:]):
            cls = 0 if a < cuts[0] else (1 if a < cuts[1] else 2)
            subs.append((w, a - lo, b - lo, cls))
    return subs


def _engine_plan(subs):
    """Greedy DVE/ACT split of the NT x len(subs) drain ops in issue order,
    using trace-fitted per-op costs (ns): DVE ~ 215+0.98*FD (incl. queue
    extras), ACT ~ 450+0.875*FD (ACTIVATE + accum-read + sem)."""
    eng = {}
    tv = ta = 0.0
    for w in range(NW):
        wsubs = [s for s in subs if s[0] == w]
        for t in range(NT):
            for (_w, s0, s1, _cls) in wsubs:
                fd = float(s1 - s0)
                cv = 215.0 + 0.98 * fd
                ca = 450.0 + 0.875 * fd
                if tv + cv <= ta + ca:
                    tv += cv
                    eng[(w, t, s0)] = "V"
                else:
                    ta += ca
                    eng[(w, t, s0)] = "A"
    return eng


def build_nc(cuts):
    """Single-core Bass program (identical on all cores; only DRAM inputs
    differ).  cuts = [c0, c0+c1] sorted-label class boundaries."""
    subs = _subranges(cuts)
    ncols = len(subs)
    assert ncols <= ACOLS, ncols
    eng = _engine_plan(subs)

    # class -> accum-column range (same structure for every d-tile)
    col_cls = [cls for (_g, _s0, _s1, cls) in subs]
    crange = []
    for k in range(K):
        idx = [i for i, cc in enumerate(col_cls) if cc == k]
        assert idx, f"class {k} empty"
        assert idx == list(range(idx[0], idx[-1] + 1))
        crange.append((idx[0], idx[-1] + 1))

    nc = bacc.Bacc()
    featT = nc.dram_tensor("featT", [C, N], MM_DT, kind="ExternalInput")
    projwT = nc.dram_tensor("projwT", [C, DLOC], MM_DT, kind="ExternalInput")
    cwadj = nc.dram_tensor("cwadj", [PT, NT * K], F32, kind="ExternalInput")
    p_out = nc.dram_tensor("p_out", [4, 2 * K, N], F32, kind="ExternalOutput")
    u_out = nc.dram_tensor("u_out", [PT, NT * K], F32, kind="ExternalOutput")

    with tile.TileContext(nc) as tc:
        with tc.tile_pool(name="singles", bufs=1) as singles, \
                tc.tile_pool(name="featp", bufs=2) as featp:
            # post the first feat chunk before the weights: its transfer is
            # the longest pole on the path to the first matmul.  fj0 is
            # persistent (singles) so the late keep-warm dummies can reuse it.
            fj0 = singles.tile([C, G4], MM_DT)
            nc.sync.dma_start(out=fj0[:, 0:WIN], in_=featT[:, 0:WIN])
            projw_sb = singles.tile([C, DLOC], MM_DT)
            nc.sync.dma_start(out=projw_sb, in_=projwT[:, :])
            nc.sync.dma_start(out=fj0[:, WIN:G4], in_=featT[:, WIN:G4])
            cwadj_sb = singles.tile([PT, NT * K], F32)
            nc.sync.dma_start(out=cwadj_sb, in_=cwadj[:, :])
            hv = [
                singles.tile([PT, N], FP8, name=f"hv{t}")
                for t in range(NT)
            ]
            sacc = singles.tile([PT, NT * ACOLS], F32)
            ssum = singles.tile([PT, NT * K], F32)
            u_sb = singles.tile([PT, NT * K], F32)
            a32 = singles.tile([PT, NT * K], F32)
            stat = singles.tile([PT, NT * SCW], FP8)
            nc.vector.memset(stat, 0.0)
            # touch Sigmoid early so ACT_TABLE_LOAD (~1.5us) runs during the
            # input DMAs instead of blocking the first real ACT drain
            nc.scalar.activation(
                sacc[:, 0:1], stat[:, 0:1],
                mybir.ActivationFunctionType.Sigmoid,
            )

            # ---- phase A: encode + step + fused segment sums ------------
            with tc.tile_pool(name="zp", bufs=4, space="PSUM") as zp:
                for g in range(NG):
                    if g == 0:
                        fj = fj0
                    else:
                        fj = featp.tile([C, G4], MM_DT, tag="fj")
                        nc.sync.dma_start(
                            out=fj, in_=featT[:, g * G4:(g + 1) * G4]
                        )
                    for t in range(NT):
                        for wi in range(G4 // WIN):
                            w = g * (G4 // WIN) + wi
                            wsubs = [
                                (s0, s1, ci)
                                for ci, (ww, s0, s1, _c) in enumerate(subs)
                                if ww == w
                            ]
                            z = zp.tile([PT, WIN], F32, tag="z")
                            for c2 in range(WIN // MMC):
                                fo = wi * WIN + c2 * MMC
                                nc.tensor.matmul(
                                    z[:, c2 * MMC:(c2 + 1) * MMC],
                                    projw_sb[:, t * PT:(t + 1) * PT],
                                    fj[:, fo:fo + MMC],
                                    start=True, stop=True,
                                )
                            for (s0, s1, ci) in wsubs:
                                dst = hv[t][:, w * WIN + s0:w * WIN + s1]
                                acc = sacc[:, t * ACOLS + ci:t * ACOLS + ci + 1]
                                if eng[(w, t, s0)] == "A":
                                    nc.scalar.activation(
                                        dst, z[:, s0:s1],
                                        mybir.ActivationFunctionType.Sigmoid,
                                        scale=SIG_SCALE,
                                        accum_out=acc,
                                    )
                                else:
                                    nc.vector.tensor_scalar(
                                        dst, z[:, s0:s1], 0.0, None,
                                        mybir.AluOpType.is_ge,
                                        mybir.AluOpType.add,
                                        accum_out=acc,
                                    )

                # keep the PE HAM-warm through the drain tail + phase B:
                # dummy matmuls into rotating zp slots (results unused) run
                # exactly when the last drains release their slots
                for _wu in range(4):
                    zw = zp.tile([PT, WIN], F32, tag="z", name=f"zwarm{_wu}")
                    nc.tensor.matmul(
                        zw[:, 0:MMC],
                        projw_sb[:, 0:PT],
                        fj0[:, 0:MMC],
                        start=True, stop=True,
                    )

            # ---- phase B: u = cwadj + step-sums; fp8 hi+res stationary --
            sacc3 = sacc.rearrange("p (t c) -> p t c", c=ACOLS)
            ssum3 = ssum.rearrange("p (t k) -> p t k", k=K)
            for k in range(K):
                a, b = crange[k]
                nc.vector.reduce_sum(
                    ssum3[:, :, k:k + 1], sacc3[:, :, a:b],
                    axis=mybir.AxisListType.X,
                )
            nc.vector.tensor_add(u_sb, ssum, cwadj_sb)
            nc.sync.dma_start(out=u_out[:, :], in_=u_sb)
            u3 = u_sb.rearrange("p (t k) -> p t k", k=K)
            a32_3 = a32.rearrange("p (t k) -> p t k", k=K)
            stat3 = stat.rearrange("p (t c) -> p t c", c=SCW)
            nc.vector.tensor_scalar(
                stat3[:, :, 0:K], u3, 0.5, None, mybir.AluOpType.mult,
            )
            nc.vector.tensor_copy(a32_3, stat3[:, :, 0:K])
            nc.vector.scalar_tensor_tensor(
                stat3[:, :, K:2 * K], u3, 0.5, a32_3,
                mybir.AluOpType.mult, mybir.AluOpType.subtract,
            )

            # ---- phase C: P partials, 4x column-tiled fp8 matmuls -------
            with (
                tc.tile_pool(name="pp", bufs=2, space="PSUM") as ppp,
                tc.tile_pool(name="pstage", bufs=3) as pstp,
            ):
                # stat-dependent keep-warm dummy: runs the moment the
                # stationaries are built, bridging the phase-B PE idle gap
                # under the 3.4us HAM window
                pqw = ppp.tile([128, PCG], F32, tag="pq")
                nc.tensor.matmul(
                    pqw[0:32, 0:PC],
                    stat[:, 0:SCW],
                    hv[0][:, 0:PC],
                    start=True, stop=True,
                    tile_position=(0, 0),
                    skip_group_check=True,
                )
                for gc in range(N // PCG):
                    pq = ppp.tile([128, PCG], F32, tag="pq")
                    for t in RR:
                        gi = GID[t]
                        for c4 in range(PCG // PC):
                            cs = slice(c4 * PC, (c4 + 1) * PC)
                            base = gc * PCG + c4 * PC
                            nc.tensor.matmul(
                                pq[32 * gi:32 * gi + 32, cs],
                                stat[:, t * SCW:(t + 1) * SCW],
                                hv[t][:, base:base + PC],
                                start=(t == FIRST[gi]),
                                stop=(t == LAST[gi]),
                                tile_position=(0, 32 * gi),
                                # the sim's group checker is zero-region
                                # (partition-blind); has_written is
                                # per-element so col groups are independent
                                skip_group_check=True,
                            )
                    pst = pstp.tile([128, PCG], F32, tag="pst")
                    h = PCG // 2
                    nc.vector.tensor_copy(pst[:, 0:h], pq[:, 0:h])
                    nc.scalar.copy(pst[:, h:PCG], pq[:, h:PCG])
                    for gi in range(4):
                        q = nc.gpsimd if gi % 2 == 0 else nc.sync
                        q.dma_start(
                            out=p_out[gi, :, gc * PCG:(gc + 1) * PCG],
                            in_=pst[32 * gi:32 * gi + 2 * K, :],
                        )
    nc.compile()
    return nc


def _prep_inputs(feat_s, proj_w, classify_weights, counts):
    featT = np.ascontiguousarray(feat_s.T).astype(np.float16)  # [128, N]
    cadj = classify_weights.astype(np.float32) \
        - 0.5 * counts[:, None].astype(np.float32)             # [K, D]
    in_maps = []
    for core in range(NCORES):
        sl = slice(core * DLOC, (core + 1) * DLOC)
        projwT = np.ascontiguousarray(proj_w[sl].T).astype(np.float16)
        ca = cadj[:, sl].T                                     # [DLOC, K]
        ca_t = np.ascontiguousarray(
            ca.reshape(NT, PT, K).transpose(1, 0, 2).reshape(PT, NT * K)
        ).astype(np.float32)
        in_maps.append({"featT": featT, "projwT": projwT, "cwadj": ca_t})
    return in_maps


def _assemble(results, perm):
    """Host: gather per-core u/P, undo the step->sign affine, normalize."""
    fp8np = mybir.dt.np(FP8)
    P = np.zeros((K, N), np.float64)
    rowsum_uq = np.zeros(K, np.float64)
    U = np.zeros((K, D), np.float32)
    for core in range(NCORES):
        r = results[core]
        u = np.asarray(r["u_out"])                             # [PT, NT*K]
        u_full = u.reshape(PT, NT, K).transpose(1, 0, 2).reshape(DLOC, K)
        U[:, core * DLOC:(core + 1) * DLOC] = u_full.T
        # replicate the device fp8 hi+res quantization exactly
        a32f = (0.5 * u_full).astype(fp8np).astype(np.float32)
        b32f = (0.5 * u_full - a32f).astype(fp8np).astype(np.float32)
        rowsum_uq += 2.0 * (a32f + b32f).astype(np.float64).sum(axis=0)
        p6 = np.asarray(r["p_out"]).astype(np.float64)         # [4, 6, N]
        P += (p6[:, 0:K] + p6[:, K:2 * K]).sum(axis=0)
    # logits2[k,n] = sum_d u_q[d,k] * (2*step - 1) = 4*P - rowsum(u_q)
    L2 = 4.0 * P - rowsum_uq[:, None]
    norms = np.linalg.norm(U.astype(np.float64), axis=1)
    logits_sorted = (L2 / np.maximum(norms, EPS)[:, None]).T.astype(np.float32)
    out = np.empty((N, K), np.float32)
    out[perm] = logits_sorted
    return out


def kernel(feat, proj_w, classify_weights, labels, _trace=False):
    global LAST_RESULTS
    feat = np.asarray(feat, dtype=np.float32)
    proj_w = np.asarray(proj_w, dtype=np.float32)
    classify_weights = np.asarray(classify_weights, dtype=np.float32)
    labels = np.asarray(labels).astype(np.int64)

    perm = np.argsort(labels, kind="stable")
    feat_s = feat[perm]
    counts = np.bincount(labels, minlength=K)
    cuts = [int(counts[0]), int(counts[0] + counts[1])]

    nc = build_nc(cuts)
    in_maps = _prep_inputs(feat_s, proj_w, classify_weights, counts)
    res = run_bass_kernel_spmd(nc, in_maps, list(range(NCORES)), trace=_trace)
    LAST_RESULTS = res
    return _assemble(res.results, perm)


# revision 16
# speedup vs baseline: 1.0118x; 1.0118x over previous
"""Trainium2 Bass kernel for the vq_codebook / HDC problem.

Math (reference):
    hv      = sign(feat @ proj_w.T)               [N=16384, D=10000], +-1 (0 -> +1)
    per_cls = segment_sum(hv, labels, K=3)        [3, D]
    updated = classify_weights + 0.5 * per_cls
    protos  = updated / max(||updated||_row, eps)
    logits  = hv @ protos.T                       [N, 3]

Strategy (8 NeuronCores, D-sharded, no collectives; ~150us HW vs 294us
baseline, rel err ~5e-3 vs 2e-2 gate):
  * Each core owns DLOC=1250 hyper-dims (10 d-tiles x 125 partitions), all
    N rows; host sorts rows by label so per-class sums become contiguous
    range sums along the free axis.
  * hv is stored as step(z) in {0,1} fp8e4 (16KB/partition per d-tile), so
    ALL 10 d-tiles stay SBUF-resident: feat is read exactly once per core
    and hv never touches HBM.  sign = 2*step - 1 is fixed up algebraically
    on the host.
  * Phase A (encode, ~103us, drain-bound): z = projw.T-tiles @ featT in
    [d, n] layout via fp16 matmuls (16-bit moving operand streams 1
    col/cycle vs ~2 for fp32r -- measured; fp16 input quantization costs
    ~4.8e-3 end-to-end rel err) into [125, 1024] PSUM window tiles, 4
    rotating 2-bank bufs so the fill<->drain WAR chain stays short.  One
    drain op per (d-tile, window) converts z -> step fp8 with fused
    per-segment accumulation (accum_out), split DVE (is_ge) / ACT
    (Sigmoid(2^20 z)) by a greedy balance on trace-fitted costs; both
    engines run concurrently on disjoint PSUM banks at ~1 elem/cyc/
    partition each -- the hard floor for this dataflow on trn2.
  * Phase B: u = (cw - 0.5*count) + step-sums == reference `updated`
    exactly; phase-C stationary = fp8(u/2) + fp8 residual per d-tile
    (zero-padded to 32 cols so every PSUM partition of a col group is
    written).  Keep-warm dummy matmuls bridge the PE idle gap so the HAM
    clock gate never re-throttles (3.4us window).
  * Phase C (consume, ~26us): P = (u_q/2) @ step via plain-fp8 matmuls
    packed 4x into the PE with column tiling (tile_position=(0,32g)): 4
    concurrent col-groups, each owning 2-3 d-tiles, stream their own
    moving operand -> ~79ns/matmul effective.  Drain halves split
    DVE/ACT; p_out DMAs alternate gpsimd/sync queues.
  * Host: logits = (4*P - rowsum(u_q)) / max(||u||, eps), un-permuted.
    u_q is re-quantized on host with ml_dtypes for an exact correction.
"""

import os
import sys

sys.path.insert(0, "/opt/trn_rl_repo")
os.environ.setdefault("MYCRO_LOCAL_CACHE", "1")

import numpy as np

import concourse.bass as bass
import concourse.tile as tile
from concourse import bacc
from concourse import mybir
from concourse.bass import MemorySpace
from concourse.bass_utils import run_bass_kernel_spmd

# ---------------------------------------------------------------- constants
N, C, D, K = 16384, 128, 10000, 3
NCORES = 8
DLOC = D // NCORES          # 1250 hyper-dims per core
PT = 125                    # partitions per d-tile
NT = DLOC // PT             # 10 d-tiles per core
G4 = 2048                   # feat DMA chunk
NG = N // G4                # 8 groups
WIN = 1024                  # phase-A window (2 PSUM banks; 4 bufs rotate)
NW = N // WIN               # 16 windows
MMC = 512                   # encode matmul chunk (PSUM bank = 512 fp32)
PC = 512                    # phase-C matmul out chunk (1 PSUM bank)
PCG = 2048                  # phase-C drain super-chunk
SIG_SCALE = 1048576.0       # 2^20: step(z) ~= Sigmoid(SIG_SCALE * z)
ACOLS = 20                  # accum columns reserved per d-tile

# phase-C column tiling: d-tile -> col group (3/3/2/2), round-robin issue
GID = [0, 0, 0, 1, 1, 1, 2, 2, 3, 3]
RR = [0, 3, 6, 8, 1, 4, 7, 9, 2, 5]
FIRST = {0: 0, 1: 3, 2: 6, 3: 8}
LAST = {0: 2, 1: 5, 2: 7, 3: 9}
SCW = 32                    # stationary cols (zero-padded past 2K so every
                            # PSUM partition in a col group is written)

MM_DT = mybir.dt.float16
FP8 = mybir.dt.float8e4
F32 = mybir.dt.float32
EPS = 1e-12

LAST_RESULTS = None         # BassKernelResults of the most recent run


def _subranges(cuts):
    """Ordered (w, s0, s1, cls) sub-ranges: each phase-A window [w*WIN,(w+1)*WIN)
    split at the sorted-label cut points so every range is single-class."""
    subs = []
    for w in range(NW):
        lo, hi = w * WIN, (w + 1) * WIN
        pts = [lo] + [c for c in cuts if lo < c < hi] + [hi]
        for a, b in zip(pts[:-1], pts[1:]):
            cls = 0 if a < cuts[0] else (1 if a < cuts[1] else 2)
            subs.append((w, a - lo, b - lo, cls))
    return subs


def _engine_plan(subs):
    """Greedy DVE/ACT split of the NT x len(subs) drain ops in issue order,
    using trace-fitted per-op costs (ns): DVE ~ 215+0.98*FD (incl. queue
    extras), ACT ~ 450+0.875*FD (ACTIVATE + accum-read + sem)."""
    eng = {}
    tv = ta = 0.0
    for w in range(NW):
        wsubs = [s for s in subs if s[0] == w]
        for t in range(NT):
            for (_w, s0, s1, _cls) in wsubs:
                fd = float(s1 - s0)
                cv = 215.0 + 0.98 * fd
                ca = 450.0 + 0.875 * fd
                if tv + cv <= ta + ca:
                    tv += cv
                    eng[(w, t, s0)] = "V"
                else:
                    ta += ca
                    eng[(w, t, s0)] = "A"
    return eng


def build_nc(cuts):
    """Single-core Bass program (identical on all cores; only DRAM inputs
    differ).  cuts = [c0, c0+c1] sorted-label class boundaries."""
    subs = _subranges(cuts)
    ncols = len(subs)
    assert ncols <= ACOLS, ncols
    eng = _engine_plan(subs)

    # class -> accum-column range (same structure for every d-tile)
    col_cls = [cls for (_g, _s0, _s1, cls) in subs]
    crange = []
    for k in range(K):
        idx = [i for i, cc in enumerate(col_cls) if cc == k]
        assert idx, f"class {k} empty"
        assert idx == list(range(idx[0], idx[-1] + 1))
        crange.append((idx[0], idx[-1] + 1))

    nc = bacc.Bacc()
    featT = nc.dram_tensor("featT", [C, N], MM_DT, kind="ExternalInput")
    projwT = nc.dram_tensor("projwT", [C, DLOC], MM_DT, kind="ExternalInput")
    cwadj = nc.dram_tensor("cwadj", [PT, NT * K], F32, kind="ExternalInput")
    p_out = nc.dram_tensor("p_out", [4, 2 * K, N], F32, kind="ExternalOutput")
    u_out = nc.dram_tensor("u_out", [PT, NT * K], F32, kind="ExternalOutput")

    with tile.TileContext(nc) as tc:
        with tc.tile_pool(name="singles", bufs=1) as singles, \
                tc.tile_pool(name="featp", bufs=2) as featp:
            # post the first feat chunk before the weights: its transfer is
            # the longest pole on the path to the first matmul.  fj0 is
            # persistent (singles) so the late keep-warm dummies can reuse it.
            # spread input DMAs across queues: posts cost ~650ns each and
            # serialize per queue; the first matmul waits on fj0h1 + projw
            fj0 = singles.tile([C, G4], MM_DT)
            nc.sync.dma_start(out=fj0[:, 0:WIN], in_=featT[:, 0:WIN])
            projw_sb = singles.tile([C, DLOC], MM_DT)
            nc.gpsimd.dma_start(out=projw_sb, in_=projwT[:, :])
            nc.scalar.dma_start(out=fj0[:, WIN:G4], in_=featT[:, WIN:G4])
            cwadj_sb = singles.tile([PT, NT * K], F32)
            nc.gpsimd.dma_start(out=cwadj_sb, in_=cwadj[:, :])
            hv = [
                singles.tile([PT, N], FP8, name=f"hv{t}")
                for t in range(NT)
            ]
            sacc = singles.tile([PT, NT * ACOLS], F32)
            ssum = singles.tile([PT, NT * K], F32)
            u_sb = singles.tile([PT, NT * K], F32)
            a32 = singles.tile([PT, NT * K], F32)
            stat = singles.tile([PT, NT * SCW], FP8)
            nc.vector.memset(stat, 0.0)
            # touch Sigmoid early so ACT_TABLE_LOAD (~1.5us) runs during the
            # input DMAs instead of blocking the first real ACT drain
            nc.scalar.activation(
                sacc[:, 0:1], stat[:, 0:1],
                mybir.ActivationFunctionType.Sigmoid,
            )

            # ---- phase A: encode + step + fused segment sums ------------
            with tc.tile_pool(name="zp", bufs=4, space="PSUM") as zp:
                for g in range(NG):
                    if g == 0:
                        fj = fj0
                    else:
                        fj = featp.tile([C, G4], MM_DT, tag="fj")
                        nc.sync.dma_start(
                            out=fj, in_=featT[:, g * G4:(g + 1) * G4]
                        )
                    for t in range(NT):
                        for wi in range(G4 // WIN):
                            w = g * (G4 // WIN) + wi
                            wsubs = [
                                (s0, s1, ci)
                                for ci, (ww, s0, s1, _c) in enumerate(subs)
                                if ww == w
                            ]
                            z = zp.tile([PT, WIN], F32, tag="z")
                            for c2 in range(WIN // MMC):
                                fo = wi * WIN + c2 * MMC
                                nc.tensor.matmul(
                                    z[:, c2 * MMC:(c2 + 1) * MMC],
                                    projw_sb[:, t * PT:(t + 1) * PT],
                                    fj[:, fo:fo + MMC],
                                    start=True, stop=True,
                                )
                            for (s0, s1, ci) in wsubs:
                                dst = hv[t][:, w * WIN + s0:w * WIN + s1]
                                acc = sacc[:, t * ACOLS + ci:t * ACOLS + ci + 1]
                                if eng[(w, t, s0)] == "A":
                                    nc.scalar.activation(
                                        dst, z[:, s0:s1],
                                        mybir.ActivationFunctionType.Sigmoid,
                                        scale=SIG_SCALE,
                                        accum_out=acc,
                                    )
                                else:
                                    nc.vector.tensor_scalar(
                                        dst, z[:, s0:s1], 0.0, None,
                                        mybir.AluOpType.is_ge,
                                        mybir.AluOpType.add,
                                        accum_out=acc,
                                    )

                # keep the PE HAM-warm through the drain tail + phase B:
                # dummy matmuls into rotating zp slots (results unused) run
                # exactly when the last drains release their slots
                for _wu in range(4):
                    zw = zp.tile([PT, WIN], F32, tag="z", name=f"zwarm{_wu}")
                    nc.tensor.matmul(
                        zw[:, 0:MMC],
                        projw_sb[:, 0:PT],
                        fj0[:, 0:MMC],
                        start=True, stop=True,
                    )

            # ---- phase B: u = cwadj + step-sums; fp8 hi+res stationary --
            sacc3 = sacc.rearrange("p (t c) -> p t c", c=ACOLS)
            ssum3 = ssum.rearrange("p (t k) -> p t k", k=K)
            for k in range(K):
                a, b = crange[k]
                nc.vector.reduce_sum(
                    ssum3[:, :, k:k + 1], sacc3[:, :, a:b],
                    axis=mybir.AxisListType.X,
                )
            nc.vector.tensor_add(u_sb, ssum, cwadj_sb)
            nc.sync.dma_start(out=u_out[:, :], in_=u_sb)
            u3 = u_sb.rearrange("p (t k) -> p t k", k=K)
            a32_3 = a32.rearrange("p (t k) -> p t k", k=K)
            stat3 = stat.rearrange("p (t c) -> p t c", c=SCW)
            nc.vector.tensor_scalar(
                stat3[:, :, 0:K], u3, 0.5, None, mybir.AluOpType.mult,
            )
            nc.vector.tensor_copy(a32_3, stat3[:, :, 0:K])
            nc.vector.scalar_tensor_tensor(
                stat3[:, :, K:2 * K], u3, 0.5, a32_3,
                mybir.AluOpType.mult, mybir.AluOpType.subtract,
            )

            # ---- phase C: P partials, 4x column-tiled fp8 matmuls -------
            with (
                tc.tile_pool(name="pp", bufs=2, space="PSUM") as ppp,
                tc.tile_pool(name="pstage", bufs=3) as pstp,
            ):
                # stat-dependent keep-warm dummy: runs the moment the
                # stationaries are built, bridging the phase-B PE idle gap
                # under the 3.4us HAM window
                pqw = ppp.tile([128, PCG], F32, tag="pq")
                nc.tensor.matmul(
                    pqw[0:32, 0:PC],
                    stat[:, 0:SCW],
                    hv[0][:, 0:PC],
                    start=True, stop=True,
                    tile_position=(0, 0),
                    skip_group_check=True,
                )
                for gc in range(N // PCG):
                    pq = ppp.tile([128, PCG], F32, tag="pq")
                    for t in RR:
                        gi = GID[t]
                        for c4 in range(PCG // PC):
                            cs = slice(c4 * PC, (c4 + 1) * PC)
                            base = gc * PCG + c4 * PC
                            nc.tensor.matmul(
                                pq[32 * gi:32 * gi + 32, cs],
                                stat[:, t * SCW:(t + 1) * SCW],
                                hv[t][:, base:base + PC],
                                start=(t == FIRST[gi]),
                                stop=(t == LAST[gi]),
                                tile_position=(0, 32 * gi),
                                # the sim's group checker is zero-region
                                # (partition-blind); has_written is
                                # per-element so col groups are independent
                                skip_group_check=True,
                            )
                    pst = pstp.tile([128, PCG], F32, tag="pst")
                    h = PCG // 2
                    nc.vector.tensor_copy(pst[:, 0:h], pq[:, 0:h])
                    nc.scalar.copy(pst[:, h:PCG], pq[:, h:PCG])
                    for gi in range(4):
                        q = nc.gpsimd if gi % 2 == 0 else nc.sync
                        q.dma_start(
                            out=p_out[gi, :, gc * PCG:(gc + 1) * PCG],
                            in_=pst[32 * gi:32 * gi + 2 * K, :],
                        )
    nc.compile()
    return nc


def _prep_inputs(feat_s, proj_w, classify_weights, counts):
    featT = np.ascontiguousarray(feat_s.T).astype(np.float16)  # [128, N]
    cadj = classify_weights.astype(np.float32) \
        - 0.5 * counts[:, None].astype(np.float32)             # [K, D]
    in_maps = []
    for core in range(NCORES):
        sl = slice(core * DLOC, (core + 1) * DLOC)
        projwT = np.ascontiguousarray(proj_w[sl].T).astype(np.float16)
        ca = cadj[:, sl].T                                     # [DLOC, K]
        ca_t = np.ascontiguousarray(
            ca.reshape(NT, PT, K).transpose(1, 0, 2).reshape(PT, NT * K)
        ).astype(np.float32)
        in_maps.append({"featT": featT, "projwT": projwT, "cwadj": ca_t})
    return in_maps


def _assemble(results, perm):
    """Host: gather per-core u/P, undo the step->sign affine, normalize."""
    fp8np = mybir.dt.np(FP8)
    P = np.zeros((K, N), np.float64)
    rowsum_uq = np.zeros(K, np.float64)
    U = np.zeros((K, D), np.float32)
    for core in range(NCORES):
        r = results[core]
        u = np.asarray(r["u_out"])                             # [PT, NT*K]
        u_full = u.reshape(PT, NT, K).transpose(1, 0, 2).reshape(DLOC, K)
        U[:, core * DLOC:(core + 1) * DLOC] = u_full.T
        # replicate the device fp8 hi+res quantization exactly
        a32f = (0.5 * u_full).astype(fp8np).astype(np.float32)
        b32f = (0.5 * u_full - a32f).astype(fp8np).astype(np.float32)
        rowsum_uq += 2.0 * (a32f + b32f).astype(np.float64).sum(axis=0)
        p6 = np.asarray(r["p_out"]).astype(np.float64)         # [4, 6, N]
        P += (p6[:, 0:K] + p6[:, K:2 * K]).sum(axis=0)
    # logits2[k,n] = sum_d u_q[d,k] * (2*step - 1) = 4*P - rowsum(u_q)
    L2 = 4.0 * P - rowsum_uq[:, None]
    norms = np.linalg.norm(U.astype(np.float64), axis=1)
    logits_sorted = (L2 / np.maximum(norms, EPS)[:, None]).T.astype(np.float32)
    out = np.empty((N, K), np.float32)
    out[perm] = logits_sorted
    return out


def kernel(feat, proj_w, classify_weights, labels, _trace=False):
    global LAST_RESULTS
    feat = np.asarray(feat, dtype=np.float32)
    proj_w = np.asarray(proj_w, dtype=np.float32)
    classify_weights = np.asarray(classify_weights, dtype=np.float32)
    labels = np.asarray(labels).astype(np.int64)

    perm = np.argsort(labels, kind="stable")
    feat_s = feat[perm]
    counts = np.bincount(labels, minlength=K)
    cuts = [int(counts[0]), int(counts[0] + counts[1])]

    nc = build_nc(cuts)
    in_maps = _prep_inputs(feat_s, proj_w, classify_weights, counts)
    res = run_bass_kernel_spmd(nc, in_maps, list(range(NCORES)), trace=_trace)
    LAST_RESULTS = res
    return _assemble(res.results, perm)
